# revision 2
# baseline (speedup 1.0000x reference)
"""Trainium2 Bass kernel for nn_GraphVToS_9388798509586 (gnn_message_passing).

Contract: kernel(**inputs) takes FULL unsharded numpy inputs and returns the
FULL [8, 128, 128, 64] float32 output.

Math (per batch element b, with F=64, K=64, C=3, N=128):
    pi = vf @ w_vs[:F]            # [N, C, K]
    pj = vf @ w_vs[F:] + b_vs     # [N, C, K]  (bias folds exactly: b*sum_c d)
    s[i,j,k] = sum_c d[i,j,c] * (pi[i,c,k] + pj[j,c,k])
    out      = relu(s)

Sharding: data-parallel over batch B=8, one batch element per NeuronCore.

Device kernel design (per core), output layout ACC[i, (j,k)]:
  * term2 (sum_c d[i,j,c]*pj[j,c,k]) runs on the TensorEngine as 8 grouped
    matmuls over j-groups of 16: contract dim (j',c)=48, lhsT = d-transpose
    chunk [48, 128i], moving = block-diagonal pj matrix [48, 16*64]. The
    block-diagonal expansion and the tiny O(N*C*K) projections are prepacked
    on the host (input relayout; all N^2-scale FLOPs stay on device).
  * term1 (sum_c d[i,j,c]*pi[i,c,k]) is j-diagonal in this layout, so it runs
    as per-j scalar_tensor_tensor MACs: per-partition-i scalars d[:,j,c]
    times pi tiles [i, k], accumulated into ACC. The 3 c-terms per j are
    split DVE/GPSIMD/DVE to balance engine load.
  * relu on the scalar engine, 512-wide; output DMA'd per group (bf16),
    upcast to f32 on the host.
"""

import numpy as np

B, N, C, F, K = 8, 128, 3, 64, 64
_N_CORES = 8
_JG = 16                 # j-group size for the term2 matmuls
_NG = N // _JG           # 8 groups
_CTR = _JG * C           # 48 contraction rows per group

_cached = {}


def _build_nc():
    import concourse.bass as bass
    import concourse.mybir as mybir
    import concourse.tile as tile

    fp32 = mybir.dt.float32
    bf16 = mybir.dt.bfloat16

    nc = bass.Bass()
    # Per-core inputs (host-prepacked, bf16).
    dsc_d = nc.dram_tensor("dsc", [N, N * C], bf16, kind="ExternalInput")
    ltw_d = nc.dram_tensor("ltw", [_CTR, _NG * N], bf16, kind="ExternalInput")
    wdg_d = nc.dram_tensor("wdg", [_CTR, _NG * _JG * K], bf16, kind="ExternalInput")
    pib_d = nc.dram_tensor("pib", [N, C * K], bf16, kind="ExternalInput")
    out_d = nc.dram_tensor("out", [N, N * K], bf16, kind="ExternalOutput")

    GW = _JG * K  # 1024 free elems per group

    with tile.TileContext(nc) as tc:
        with (
            tc.tile_pool(name="persist", bufs=1) as pp,
            tc.tile_pool(name="acc", bufs=3) as ap,
            tc.tile_pool(name="psum", bufs=3, space="PSUM") as qp,
        ):
            dsc = pp.tile([N, N * C], bf16, tag="dsc")
            ltw = pp.tile([_CTR, _NG * N], bf16, tag="ltw")
            pib = pp.tile([N, C * K], bf16, tag="pib")
            wdg = pp.tile([_CTR, _NG * GW], bf16, tag="wdg")

            nc.sync.dma_start(dsc[:], dsc_d[:])
            nc.sync.dma_start(ltw[:], ltw_d[:])
            nc.sync.dma_start(pib[:], pib_d[:])
            # wdiag arrives per group so group 0 can start early.
            for g in range(_NG):
                nc.sync.dma_start(
                    wdg[:, g * GW : (g + 1) * GW],
                    wdg_d[:, g * GW : (g + 1) * GW],
                )

            for g in range(_NG):
                ps = qp.tile([N, GW], fp32, tag="ps")
                acc = ap.tile([N, GW], bf16, tag="acc")
                lhsT = ltw[:, g * N : (g + 1) * N]
                # Two N=512 matmuls sharing one stationary load.
                for h in range(2):
                    nc.tensor.matmul(
                        ps[:, h * 512 : (h + 1) * 512],
                        lhsT,
                        wdg[:, g * GW + h * 512 : g * GW + (h + 1) * 512],
                        start=True,
                        stop=True,
                    )
                # term2 -> bf16 ACC
                nc.vector.tensor_copy(acc[:], ps[:])
                # term1: 3 per-partition-scalar MACs per j (DVE, GPSIMD, DVE)
                for jg in range(_JG):
                    j = g * _JG + jg
                    sl = acc[:, jg * K : (jg + 1) * K]
                    for c, eng in ((0, nc.vector), (1, nc.gpsimd), (2, nc.vector)):
                        eng.scalar_tensor_tensor(
                            sl,
                            pib[:, c * K : (c + 1) * K],
                            dsc[:, 3 * j + c : 3 * j + c + 1],
                            sl,
                            mybir.AluOpType.mult,
                            mybir.AluOpType.add,
                        )
                # relu (in place, 512-wide) then store
                for h in range(2):
                    sl = acc[:, h * 512 : (h + 1) * 512]
                    nc.scalar.activation(sl, sl, mybir.ActivationFunctionType.Relu)
                nc.sync.dma_start(out_d[:, g * GW : (g + 1) * GW], acc[:])
    return nc


def _host_pack(vf, d, w, b):
    """Per-batch host prepack -> list of per-core input dicts (bf16)."""
    import ml_dtypes

    bf = ml_dtypes.bfloat16
    w_i, w_j = w[:F], w[F:]
    # [B, N, C, K] projections (tiny: O(N*C*F*K))
    pi = np.einsum("bncf,fk->bnck", vf, w_i, optimize=True)
    pj = np.einsum("bncf,fk->bnck", vf, w_j, optimize=True) + b

    in_maps = []
    for bi in range(B):
        dsc = np.ascontiguousarray(d[bi].reshape(N, N * C)).astype(bf)
        # ltw[(j%16)*3+c, g*128+i] = d[i, j, c]
        A = d[bi].transpose(1, 2, 0).reshape(N * C, N)  # [(j,c), i]
        ltw = (
            A.reshape(_NG, _CTR, N).transpose(1, 0, 2).reshape(_CTR, _NG * N)
        ).astype(bf)
        # wdiag[(j%16)*3+c, g*1024 + (j%16)*64 + k] = pj[j, c, k]
        wdg = np.zeros((_CTR, _NG, _JG, K), np.float32)
        jj = np.arange(_JG)
        for c in range(C):
            wdg[jj * C + c, :, jj, :] = (
                pj[bi].reshape(_NG, _JG, C, K)[:, jj, c, :].transpose(1, 0, 2)
            )
        wdg = np.ascontiguousarray(wdg.reshape(_CTR, _NG * _JG * K)).astype(bf)
        pib = np.ascontiguousarray(pi[bi].reshape(N, C * K)).astype(bf)
        in_maps.append({"dsc": dsc, "ltw": ltw, "wdg": wdg, "pib": pib})
    return in_maps


def _run(in_maps, trace=False, **kw):
    from concourse.bass_utils import run_bass_kernel_spmd

    if "nc" not in _cached:
        _cached["nc"] = _build_nc()
    return run_bass_kernel_spmd(
        _cached["nc"], in_maps, core_ids=list(range(_N_CORES)), trace=trace, **kw
    )


def kernel(**inputs: np.ndarray) -> np.ndarray:
    vf = np.asarray(inputs["vector_features"], np.float32)
    d = np.asarray(inputs["distances"], np.float32)
    w = np.asarray(inputs["w_vs"], np.float32)
    b = np.asarray(inputs["b_vs"], np.float32)

    in_maps = _host_pack(vf, d, w, b)
    res = _run(in_maps)
    out = np.stack([r["out"] for r in res.results])  # [B, N, N*K] bf16
    return out.reshape(B, N, N, K).astype(np.float32)


if __name__ == "__main__":
    rng = np.random.default_rng(0)
    ins = {
        "vector_features": rng.standard_normal((B, N, C, F)).astype(np.float32),
        "distances": rng.standard_normal((B, N, N, C)).astype(np.float32),
        "w_vs": (rng.standard_normal((2 * F, K)) / np.sqrt(2 * F)).astype(np.float32),
        "b_vs": np.zeros(K, np.float32),
    }
    out = kernel(**ins)
    # local numpy check
    pi = np.einsum("bncf,fk->bnck", ins["vector_features"], ins["w_vs"][:F])
    pj = np.einsum("bncf,fk->bnck", ins["vector_features"], ins["w_vs"][F:])
    s = np.einsum("bick,bijc->bijk", pi, ins["distances"]) + np.einsum(
        "bjck,bijc->bijk", pj, ins["distances"]
    )
    want = np.maximum(s, 0)
    rel = np.abs(out - want).max() / np.abs(want).max()
    print("rel err vs numpy:", rel)


# revision 19
# speedup vs baseline: 652.3210x; 652.3210x over previous
"""Trainium2 Bass kernel for nn_GraphVToS_9388798509586 (gnn_message_passing).

Contract: kernel(**inputs) takes FULL unsharded numpy inputs and returns the
FULL [8, 128, 128, 64] float32 output.

Math (per batch element b, with F=64, K=64, C=3, N=128):
    pi = vf @ w_vs[:F]            # [N, C, K]
    pj = vf @ w_vs[F:] + b_vs     # [N, C, K]  (bias folds exactly: b*sum_c d)
    s[i,j,k] = sum_c d[i,j,c] * (pi[i,c,k] + pj[j,c,k])
    out      = relu(s)

Sharding: data-parallel over batch B=8, one batch element per NeuronCore.

Device kernel design (per core), output layout ACC[i, (j,k)]:
  * term2 (sum_c d[i,j,c]*pj[j,c,k]) runs on the TensorEngine as 8 grouped
    matmuls over j-groups of 16: contract dim (j',c)=48, lhsT = d-transpose
    chunk [48, 128i], moving = block-diagonal pj matrix [48, 16*64]. The
    block-diagonal expansion and the tiny O(N*C*K) projections are prepacked
    on the host (input relayout; all N^2-scale FLOPs stay on device).
  * term1 (sum_c d[i,j,c]*pi[i,c,k]) is j-diagonal in this layout, so it runs
    as per-j scalar_tensor_tensor MACs: per-partition-i scalars d[:,j,c]
    times pi tiles [i, k], accumulated into ACC. The 3 c-terms per j are
    split DVE/GPSIMD/DVE to balance engine load.
  * relu on the scalar engine, 512-wide; output DMA'd per group (bf16),
    upcast to f32 on the host.
"""

import numpy as np

B, N, C, F, K = 8, 128, 3, 64, 64
_N_CORES = 8
_JG = 16                 # j-group size for the term2 matmuls
_NG = N // _JG           # 8 groups
_CTR = _JG * C           # 48 contraction rows per group

_cached = {}


def _build_nc():
    import concourse.mybir as mybir
    import concourse.tile as tile
    from concourse import bacc

    fp32 = mybir.dt.float32
    bf16 = mybir.dt.bfloat16

    # Bacc (not plain Bass): its finalize() runs the TRN2 sync legalization
    # (each instruction may carry at most one semaphore wait).
    nc = bacc.Bacc(None)
    # Per-core inputs (host-prepacked, bf16), packed into two DRAM tensors so
    # every consumer waits on at most one DMA semaphore lane (the STT ISA
    # struct only has room for 2 sync waits).
    d128_d = nc.dram_tensor("d128", [N, N * C + C * K], bf16, kind="ExternalInput")
    d48_d = nc.dram_tensor(
        "d48", [_CTR, _NG * N + _NG * _JG * K], bf16, kind="ExternalInput"
    )
    out_d = nc.dram_tensor("out", [N, N * K], bf16, kind="ExternalOutput")

    GW = _JG * K  # 1024 free elems per group

    with tile.TileContext(nc) as tc:
        with (
            tc.tile_pool(name="persist", bufs=1) as pp,
            tc.tile_pool(name="acc", bufs=8) as ap,
            tc.tile_pool(name="psum", bufs=8, space="PSUM") as qp,
        ):
            d128 = pp.tile([N, N * C + C * K], bf16, tag="d128")
            d128s = pp.tile([N, N * C + C * K], bf16, tag="d128s")
            ob = pp.tile([N, N * K], bf16, tag="ob")
            d48 = pp.tile([_CTR, _NG * N + _NG * GW], bf16, tag="d48")
            ltw = d48[:, 0 : _NG * N]
            wdg = d48[:, _NG * N : _NG * N + _NG * GW]

            nc.sync.dma_start(d128[:], d128_d[:])
            nc.sync.dma_start(d48[:], d48_d[:])
            # Stage through DVE: downstream stt ops then depend only on the
            # DVE semaphore (the STT ISA struct fits a single sync wait).
            nc.vector.tensor_copy(d128s[:], d128[:])
            dsc = d128s[:, 0 : N * C]
            pib = d128s[:, N * C : N * C + C * K]

            for g in range(_NG):
                acc = ap.tile([N, GW], bf16, tag="acc")
                lhsT = ltw[:, g * N : (g + 1) * N]
                # Two N=512 matmuls sharing one stationary load, each into
                # its own single-bank PSUM tile.
                for h in range(2):
                    ps = qp.tile([N, 512], fp32, tag="ps")
                    nc.tensor.matmul(
                        ps[:],
                        lhsT,
                        wdg[:, g * GW + h * 512 : g * GW + (h + 1) * 512],
                        start=True,
                        stop=True,
                    )
                    # term2 -> bf16 ACC
                    nc.vector.tensor_copy(acc[:, h * 512 : (h + 1) * 512], ps[:])
                # term1: 3 per-partition-scalar MACs per j (DVE, GPSIMD, DVE)
                for jg in range(_JG):
                    j = g * _JG + jg
                    sl = acc[:, jg * K : (jg + 1) * K]
                    for c, eng in ((0, nc.vector), (1, nc.vector), (2, nc.vector)):
                        eng.scalar_tensor_tensor(
                            sl,
                            pib[:, c * K : (c + 1) * K],
                            dsc[:, 3 * j + c : 3 * j + c + 1],
                            sl,
                            mybir.AluOpType.mult,
                            mybir.AluOpType.add,
                        )
                # relu into the static output staging buffer, then store every
                # 2 groups (6 DMAs total keeps every DMA on a fresh HW lane).
                for h in range(2):
                    nc.scalar.activation(
                        ob[:, g * GW + h * 512 : g * GW + (h + 1) * 512],
                        acc[:, h * 512 : (h + 1) * 512],
                        mybir.ActivationFunctionType.Relu,
                    )
                if g % 4 == 3:
                    nc.sync.dma_start(
                        out_d[:, (g - 3) * GW : (g + 1) * GW],
                        ob[:, (g - 3) * GW : (g + 1) * GW],
                    )
    nc.finalize()
    return nc


def _host_pack(vf, d, w, b):
    """Per-batch host prepack -> list of per-core input dicts (bf16)."""
    import ml_dtypes

    bf = ml_dtypes.bfloat16
    w_i, w_j = w[:F], w[F:]
    # [B, N, C, K] projections (tiny: O(N*C*F*K))
    pi = np.einsum("bncf,fk->bnck", vf, w_i, optimize=True)
    pj = np.einsum("bncf,fk->bnck", vf, w_j, optimize=True) + b

    in_maps = []
    for bi in range(B):
        dsc = np.ascontiguousarray(d[bi].reshape(N, N * C)).astype(bf)
        # ltw[(j%16)*3+c, g*128+i] = d[i, j, c]
        A = d[bi].transpose(1, 2, 0).reshape(N * C, N)  # [(j,c), i]
        ltw = (
            A.reshape(_NG, _CTR, N).transpose(1, 0, 2).reshape(_CTR, _NG * N)
        ).astype(bf)
        # wdiag[(j%16)*3+c, g*1024 + (j%16)*64 + k] = pj[j, c, k]
        wdg = np.zeros((_CTR, _NG, _JG, K), np.float32)
        jj = np.arange(_JG)
        for c in range(C):
            wdg[jj * C + c, :, jj, :] = (
                pj[bi].reshape(_NG, _JG, C, K)[:, jj, c, :].transpose(1, 0, 2)
            )
        wdg = np.ascontiguousarray(wdg.reshape(_CTR, _NG * _JG * K)).astype(bf)
        pib = np.ascontiguousarray(pi[bi].reshape(N, C * K)).astype(bf)
        d128 = np.ascontiguousarray(np.concatenate([dsc, pib], axis=1))
        d48 = np.ascontiguousarray(np.concatenate([ltw, wdg], axis=1))
        in_maps.append({"d128": d128, "d48": d48})
    return in_maps


def _run(in_maps, trace=False, **kw):
    from concourse.bass_utils import run_bass_kernel_spmd

    if "nc" not in _cached:
        _cached["nc"] = _build_nc()
    return run_bass_kernel_spmd(
        _cached["nc"], in_maps, core_ids=list(range(_N_CORES)), trace=trace, **kw
    )


def kernel(**inputs: np.ndarray) -> np.ndarray:
    vf = np.asarray(inputs["vector_features"], np.float32)
    d = np.asarray(inputs["distances"], np.float32)
    w = np.asarray(inputs["w_vs"], np.float32)
    b = np.asarray(inputs["b_vs"], np.float32)

    in_maps = _host_pack(vf, d, w, b)
    res = _run(in_maps)
    out = np.stack([r["out"] for r in res.results])  # [B, N, N*K] bf16
    return out.reshape(B, N, N, K).astype(np.float32)


if __name__ == "__main__":
    rng = np.random.default_rng(0)
    ins = {
        "vector_features": rng.standard_normal((B, N, C, F)).astype(np.float32),
        "distances": rng.standard_normal((B, N, N, C)).astype(np.float32),
        "w_vs": (rng.standard_normal((2 * F, K)) / np.sqrt(2 * F)).astype(np.float32),
        "b_vs": np.zeros(K, np.float32),
    }
    out = kernel(**ins)
    # local numpy check
    pi = np.einsum("bncf,fk->bnck", ins["vector_features"], ins["w_vs"][:F])
    pj = np.einsum("bncf,fk->bnck", ins["vector_features"], ins["w_vs"][F:])
    s = np.einsum("bick,bijc->bijk", pi, ins["distances"]) + np.einsum(
        "bjck,bijc->bijk", pj, ins["distances"]
    )
    want = np.maximum(s, 0)
    rel = np.abs(out - want).max() / np.abs(want).max()
    print("rel err vs numpy:", rel)


# revision 21
# speedup vs baseline: 1035.9573x; 1.5881x over previous
"""Trainium2 Bass kernel for nn_GraphVToS_9388798509586 (gnn_message_passing).

Contract: kernel(**inputs) takes FULL unsharded numpy inputs and returns the
FULL [8, 128, 128, 64] float32 output.

Math (per batch element b, with F=64, K=64, C=3, N=128):
    pi = vf @ w_vs[:F]            # [N, C, K]
    pj = vf @ w_vs[F:] + b_vs     # [N, C, K]  (bias folds exactly: b*sum_c d)
    s[i,j,k] = sum_c d[i,j,c] * (pi[i,c,k] + pj[j,c,k])
    out      = relu(s)

Sharding: data-parallel over batch B=8, one batch element per NeuronCore.

Device kernel design (per core). Both terms run on the TensorEngine as
grouped block-diagonal matmuls over groups of 16 rows (contract dim
(row,c)=48), with the tiny O(N*C*K) projections and the block-diagonal
weight expansion prepacked on the host (pure input relayout; all N^2-scale
FLOPs stay on device):

  * term2[i,(j,k)] = sum_c d[i,j,c]*pj[j,c,k]: lhsT = d^T chunk [48, 128i],
    moving = block-diag pj [48, 16*64]. Output lands directly in the final
    [i, (j,k)] layout.
  * term1[j,(i,k)] = sum_c d[i,j,c]*pi[i,c,k] is only a matmul in the
    TRANSPOSED layout (j on partitions). It is computed there, then the
    (i<->j) layout fix goes through a DRAM scratch round-trip: contiguous
    per-partition scatter [j,(i,k)] -> scratch[j,i,k], then strided
    readback [i,(j,k)] (128B chunks). DMA through DRAM is the only
    cross-partition reorder path that doesn't burn compute-engine time.
  * combine: DVE adds term2-psum + term1-readback into bf16, ACT relu,
    contiguous output DMA. Output is bf16, upcast to f32 on the host
    (rel-err budget 2e-2; measured ~6e-3).
"""

import numpy as np

B, N, C, F, K = 8, 128, 3, 64, 64
_N_CORES = 8
_G = 16                  # group size (rows per block-diag group)
_NG = N // _G            # 8 groups
_CTR = _G * C            # 48 contraction rows per group

_cached = {}


def _build_nc():
    import concourse.mybir as mybir
    import concourse.tile as tile
    from concourse import bacc

    fp32 = mybir.dt.float32
    bf16 = mybir.dt.bfloat16

    GW = _G * K  # 1024 free elems per group
    LTW = _NG * N  # 1024 cols of d-transpose chunks
    WDW = _NG * GW  # 8192 cols of block-diag weights

    # Bacc (not plain Bass): its finalize() runs the TRN2 sync legalization
    # (each instruction may carry at most one raw semaphore wait).
    nc = bacc.Bacc(None)
    # One packed input tensor: [ ltw2 | wdg2 ] x { term2(pj), term1(pi) }.
    d48_d = nc.dram_tensor("d48", [_CTR, 2 * (LTW + WDW)], bf16, kind="ExternalInput")
    out_d = nc.dram_tensor("out", [N, N * K], bf16, kind="ExternalOutput")
    scr_d = nc.dram_tensor("scr", [N, N * K], bf16, kind="Internal")

    with tile.TileContext(nc) as tc:
        with (
            tc.tile_pool(name="persist", bufs=1) as pp,
            tc.tile_pool(name="acc", bufs=8) as ap,
            tc.tile_pool(name="rb", bufs=8) as rp,
            tc.tile_pool(name="psum", bufs=8, space="PSUM") as qp,
        ):
            d48 = pp.tile([_CTR, 2 * (LTW + WDW)], bf16, tag="d48")
            t1 = pp.tile([N, N * K], bf16, tag="t1")
            ob = pp.tile([N, N * K], bf16, tag="ob")
            ltw2 = d48[:, 0:LTW]
            wdg2 = d48[:, LTW : LTW + WDW]
            ltw1 = d48[:, LTW + WDW : 2 * LTW + WDW]
            wdg1 = d48[:, 2 * LTW + WDW : 2 * (LTW + WDW)]

            for h in range(2):
                w = (LTW + WDW)
                nc.sync.dma_start(d48[:, h * w : (h + 1) * w], d48_d[:, h * w : (h + 1) * w])

            # Phase A: term1 in transposed layout [j, (i,k)] -> t1 (bf16).
            for g in range(_NG):
                for h in range(2):
                    ps = qp.tile([N, 512], fp32, tag="ps")
                    nc.tensor.matmul(
                        ps[:],
                        ltw1[:, g * N : (g + 1) * N],
                        wdg1[:, g * GW + h * 512 : g * GW + (h + 1) * 512],
                        start=True,
                        stop=True,
                    )
                    nc.vector.tensor_copy(
                        t1[:, g * GW + h * 512 : g * GW + (h + 1) * 512], ps[:]
                    )

            # Phase B: scatter t1 -> scratch[j, i, k] (contiguous per
            # partition), 2 DMAs.
            for h in range(2):
                nc.sync.dma_start(
                    scr_d[:, h * 4 * GW : (h + 1) * 4 * GW],
                    t1[:, h * 4 * GW : (h + 1) * 4 * GW],
                )

            # Phase C: per group g: strided readback [i, (j16,k)] of
            # scratch rows j in group g; term2 matmuls; combine; relu; store.
            # scratch element (j, i, k) at flat offset (j*8192 + i*64 + k);
            # readback block wants order (i, j, k).
            scr_flat = scr_d.rearrange("a b -> (a b)")
            for g in range(_NG):
                rb = rp.tile([N, GW], bf16, tag="rb")
                rbv = rb.rearrange("a (j k) -> a j k", j=_G)
                src = scr_flat.rearrange(
                    "(j i k) -> i j k", j=N, i=N, k=K
                )[:, g * _G : (g + 1) * _G, :]
                nc.sync.dma_start(rbv[:], src)

                acc = ap.tile([N, GW], bf16, tag="acc")
                for h in range(2):
                    ps = qp.tile([N, 512], fp32, tag="ps")
                    nc.tensor.matmul(
                        ps[:],
                        ltw2[:, g * N : (g + 1) * N],
                        wdg2[:, g * GW + h * 512 : g * GW + (h + 1) * 512],
                        start=True,
                        stop=True,
                    )
                    nc.vector.tensor_tensor(
                        acc[:, h * 512 : (h + 1) * 512],
                        ps[:],
                        rb[:, h * 512 : (h + 1) * 512],
                        mybir.AluOpType.add,
                    )
                for h in range(2):
                    nc.scalar.activation(
                        ob[:, g * GW + h * 512 : g * GW + (h + 1) * 512],
                        acc[:, h * 512 : (h + 1) * 512],
                        mybir.ActivationFunctionType.Relu,
                    )
                if g % 4 == 3:
                    nc.sync.dma_start(
                        out_d[:, (g - 3) * GW : (g + 1) * GW],
                        ob[:, (g - 3) * GW : (g + 1) * GW],
                    )
    nc.finalize()
    return nc


def _host_pack(vf, d, w, b):
    """Per-batch host prepack -> list of per-core input dicts (bf16)."""
    import ml_dtypes

    bf = ml_dtypes.bfloat16
    w_i, w_j = w[:F], w[F:]
    # [B, N, C, K] projections (tiny: O(N*C*F*K))
    pi = np.einsum("bncf,fk->bnck", vf, w_i, optimize=True)
    pj = np.einsum("bncf,fk->bnck", vf, w_j, optimize=True) + b

    def pack_side(dT, proj):
        # dT: [N_out_rows, N_cols...] -> ltw[(r%16)*3+c, g*128+col] = d-val
        # dT is [rows, c, cols] with rows grouped by 16.
        ltw = dT.reshape(_NG, _CTR, N).transpose(1, 0, 2).reshape(_CTR, _NG * N)
        wdg = np.zeros((_CTR, _NG, _G, K), np.float32)
        rr = np.arange(_G)
        for c in range(C):
            wdg[rr * C + c, :, rr, :] = (
                proj.reshape(_NG, _G, C, K)[:, rr, c, :].transpose(1, 0, 2)
            )
        return ltw, wdg.reshape(_CTR, _NG * _G * K)

    in_maps = []
    for bi in range(B):
        # term2: rows = j, matmul output partitions = i
        A2 = d[bi].transpose(1, 2, 0).reshape(N * C, N)  # [(j,c), i]
        ltw2, wdg2 = pack_side(A2, pj[bi])
        # term1: rows = i, matmul output partitions = j
        A1 = d[bi].transpose(0, 2, 1).reshape(N * C, N)  # [(i,c), j]
        ltw1, wdg1 = pack_side(A1, pi[bi])
        d48 = np.ascontiguousarray(
            np.concatenate([ltw2, wdg2, ltw1, wdg1], axis=1)
        ).astype(bf)
        in_maps.append({"d48": d48})
    return in_maps


def _run(in_maps, trace=False, **kw):
    from concourse.bass_utils import run_bass_kernel_spmd

    if "nc" not in _cached:
        _cached["nc"] = _build_nc()
    return run_bass_kernel_spmd(
        _cached["nc"], in_maps, core_ids=list(range(_N_CORES)), trace=trace, **kw
    )


def kernel(**inputs: np.ndarray) -> np.ndarray:
    vf = np.asarray(inputs["vector_features"], np.float32)
    d = np.asarray(inputs["distances"], np.float32)
    w = np.asarray(inputs["w_vs"], np.float32)
    b = np.asarray(inputs["b_vs"], np.float32)

    in_maps = _host_pack(vf, d, w, b)
    res = _run(in_maps)
    out = np.stack([r["out"] for r in res.results])  # [B, N, N*K] bf16
    return out.reshape(B, N, N, K).astype(np.float32)


if __name__ == "__main__":
    rng = np.random.default_rng(0)
    ins = {
        "vector_features": rng.standard_normal((B, N, C, F)).astype(np.float32),
        "distances": rng.standard_normal((B, N, N, C)).astype(np.float32),
        "w_vs": (rng.standard_normal((2 * F, K)) / np.sqrt(2 * F)).astype(np.float32),
        "b_vs": np.zeros(K, np.float32),
    }
    out = kernel(**ins)
    pi = np.einsum("bncf,fk->bnck", ins["vector_features"], ins["w_vs"][:F])
    pj = np.einsum("bncf,fk->bnck", ins["vector_features"], ins["w_vs"][F:])
    s = np.einsum("bick,bijc->bijk", pi, ins["distances"]) + np.einsum(
        "bjck,bijc->bijk", pj, ins["distances"]
    )
    want = np.maximum(s, 0)
    rel = np.abs(out - want).max() / np.abs(want).max()
    print("rel err vs numpy:", rel)


# revision 24
# speedup vs baseline: 1438.3322x; 1.3884x over previous
"""Trainium2 Bass kernel for nn_GraphVToS_9388798509586 (gnn_message_passing).

Contract: kernel(**inputs) takes FULL unsharded numpy inputs and returns the
FULL [8, 128, 128, 64] float32 output.

Math (per batch element b, with F=64, K=64, C=3, N=128):
    pi = vf @ w_vs[:F]            # [N, C, K]
    pj = vf @ w_vs[F:] + b_vs     # [N, C, K]  (bias folds exactly: b*sum_c d)
    s[i,j,k] = sum_c d[i,j,c] * (pi[i,c,k] + pj[j,c,k])
    out      = relu(s)

Sharding: data-parallel over batch B=8, one batch element per NeuronCore.

Device kernel design (per core). Both terms run on the TensorEngine as
grouped block-diagonal matmuls over groups of 16 rows (contract dim
(row,c)=48), with the tiny O(N*C*K) projections and the block-diagonal
weight expansion prepacked on the host (pure input relayout; all N^2-scale
FLOPs stay on device):

  * term2[i,(j,k)] = sum_c d[i,j,c]*pj[j,c,k]: lhsT = d^T chunk [48, 128i],
    moving = block-diag pj [48, 16*64]. Output lands directly in the final
    [i, (j,k)] layout.
  * term1[j,(i,k)] = sum_c d[i,j,c]*pi[i,c,k] is only a matmul in the
    TRANSPOSED layout (j on partitions). It is computed there, then the
    (i<->j) layout fix goes through a DRAM scratch round-trip: contiguous
    per-partition scatter [j,(i,k)] -> scratch[j,i,k], then strided
    readback [i,(j,k)] (128B chunks). DMA through DRAM is the only
    cross-partition reorder path that doesn't burn compute-engine time.
  * combine: DVE adds term2-psum + term1-readback into bf16, ACT relu,
    contiguous output DMA. Output is bf16, upcast to f32 on the host
    (rel-err budget 2e-2; measured ~6e-3).
"""

import numpy as np

B, N, C, F, K = 8, 128, 3, 64, 64
_N_CORES = 8
_G = 16                  # group size (rows per block-diag group)
_NG = N // _G            # 8 groups
_CTR = _G * C            # 48 contraction rows per group

_cached = {}


def _build_nc():
    import concourse.mybir as mybir
    import concourse.tile as tile
    from concourse import bacc

    fp32 = mybir.dt.float32
    bf16 = mybir.dt.bfloat16

    GW = _G * K  # 1024 free elems per group
    LTW = _NG * N  # 1024 cols of d-transpose chunks
    WDW = _NG * GW  # 8192 cols of block-diag weights

    # Bacc (not plain Bass): its finalize() runs the TRN2 sync legalization
    # (each instruction may carry at most one raw semaphore wait).
    nc = bacc.Bacc(None)
    # Packed input: one [128, 9216] tensor hitting all DMA ports.
    # Rows 0:48   = [ltw2 | wdg2]  (term2: output partitions = i)
    # Rows 64:112 = [ltw1 | wdg1]  (term1: output partitions = j)
    # 64 is a legal quadrant base for a 48-row matmul operand.
    W = LTW + WDW  # 9216
    din_d = nc.dram_tensor("din", [N, W], bf16, kind="ExternalInput")
    out_d = nc.dram_tensor("out", [N, N * K], bf16, kind="ExternalOutput")
    scr_d = nc.dram_tensor("scr", [N, N * K], bf16, kind="Internal")

    with tile.TileContext(nc) as tc:
        with (
            tc.tile_pool(name="persist", bufs=1) as pp,
            tc.tile_pool(name="acc", bufs=8) as ap,
            tc.tile_pool(name="rb", bufs=8) as rp,
            tc.tile_pool(name="psum", bufs=8, space="PSUM") as qp,
        ):
            din = pp.tile([N, W], bf16, tag="din")
            ltw2 = din[0:_CTR, 0:LTW]
            wdg2 = din[0:_CTR, LTW:W]
            ltw1 = din[64 : 64 + _CTR, 0:LTW]
            wdg1 = din[64 : 64 + _CTR, LTW:W]

            # 4 column-chunk input DMAs so early groups start quickly.
            # Chunk 0 covers all of ltw (1024 cols) + first wdg cols.
            for h in range(4):
                cw = W // 4
                nc.sync.dma_start(
                    din[:, h * cw : (h + 1) * cw], din_d[:, h * cw : (h + 1) * cw]
                )

            # Phase A: term1 in transposed layout [j, (i,k)] -> t1 (bf16).
            # Casts split DVE/ACT to halve the per-engine load; scatter to
            # scratch[j, i, k] in 2 DMAs as soon as each half of t1 lands.
            t1 = pp.tile([N, N * K], bf16, tag="t1")
            for g in range(_NG):
                for h in range(2):
                    ps = qp.tile([N, 512], fp32, tag="ps")
                    nc.tensor.matmul(
                        ps[:],
                        ltw1[:, g * N : (g + 1) * N],
                        wdg1[:, g * GW + h * 512 : g * GW + (h + 1) * 512],
                        start=True,
                        stop=True,
                    )
                    sl = t1[:, g * GW + h * 512 : g * GW + (h + 1) * 512]
                    if h == 0:
                        nc.vector.tensor_copy(sl, ps[:])
                    else:
                        nc.scalar.copy(sl, ps[:])
                if g % 4 == 3:
                    nc.sync.dma_start(
                        scr_d[:, (g - 3) * GW : (g + 1) * GW],
                        t1[:, (g - 3) * GW : (g + 1) * GW],
                    )

            # Phase C: per group g: strided readback [i, (j16,k)] of scratch
            # rows j in group g; term2 matmuls; combine; relu; store.
            scr_flat = scr_d.rearrange("a b -> (a b)")
            for g in range(_NG):
                rb = rp.tile([N, GW], bf16, tag="rb")
                rbv = rb.rearrange("a (j k) -> a j k", j=_G)
                src = scr_flat.rearrange(
                    "(j i k) -> i j k", j=N, i=N, k=K
                )[:, g * _G : (g + 1) * _G, :]
                nc.sync.dma_start(rbv[:], src)

                acc = ap.tile([N, GW], bf16, tag="acc")
                for h in range(2):
                    ps = qp.tile([N, 512], fp32, tag="ps")
                    nc.tensor.matmul(
                        ps[:],
                        ltw2[:, g * N : (g + 1) * N],
                        wdg2[:, g * GW + h * 512 : g * GW + (h + 1) * 512],
                        start=True,
                        stop=True,
                    )
                    nc.vector.tensor_tensor(
                        acc[:, h * 512 : (h + 1) * 512],
                        ps[:],
                        rb[:, h * 512 : (h + 1) * 512],
                        mybir.AluOpType.add,
                    )
                for h in range(2):
                    sl = acc[:, h * 512 : (h + 1) * 512]
                    nc.scalar.activation(sl, sl, mybir.ActivationFunctionType.Relu)
                nc.sync.dma_start(out_d[:, g * GW : (g + 1) * GW], acc[:])
    nc.finalize()
    return nc


def _host_pack(vf, d, w, b):
    """Per-batch host prepack -> list of per-core input dicts (bf16)."""
    import ml_dtypes

    bf = ml_dtypes.bfloat16
    w_i, w_j = w[:F], w[F:]
    # [B, N, C, K] projections (tiny: O(N*C*F*K))
    pi = np.einsum("bncf,fk->bnck", vf, w_i, optimize=True)
    pj = np.einsum("bncf,fk->bnck", vf, w_j, optimize=True) + b

    def pack_side(dT, proj):
        # dT: [N_out_rows, N_cols...] -> ltw[(r%16)*3+c, g*128+col] = d-val
        # dT is [rows, c, cols] with rows grouped by 16.
        ltw = dT.reshape(_NG, _CTR, N).transpose(1, 0, 2).reshape(_CTR, _NG * N)
        wdg = np.zeros((_CTR, _NG, _G, K), np.float32)
        rr = np.arange(_G)
        for c in range(C):
            wdg[rr * C + c, :, rr, :] = (
                proj.reshape(_NG, _G, C, K)[:, rr, c, :].transpose(1, 0, 2)
            )
        return ltw, wdg.reshape(_CTR, _NG * _G * K)

    in_maps = []
    for bi in range(B):
        # term2: rows = j, matmul output partitions = i
        A2 = d[bi].transpose(1, 2, 0).reshape(N * C, N)  # [(j,c), i]
        ltw2, wdg2 = pack_side(A2, pj[bi])
        # term1: rows = i, matmul output partitions = j
        A1 = d[bi].transpose(0, 2, 1).reshape(N * C, N)  # [(i,c), j]
        ltw1, wdg1 = pack_side(A1, pi[bi])
        din = np.zeros((N, ltw2.shape[1] + wdg2.shape[1]), np.float32)
        din[0:_CTR] = np.concatenate([ltw2, wdg2], axis=1)
        din[64 : 64 + _CTR] = np.concatenate([ltw1, wdg1], axis=1)
        in_maps.append({"din": np.ascontiguousarray(din).astype(bf)})
    return in_maps


def _run(in_maps, trace=False, **kw):
    from concourse.bass_utils import run_bass_kernel_spmd

    if "nc" not in _cached:
        _cached["nc"] = _build_nc()
    return run_bass_kernel_spmd(
        _cached["nc"], in_maps, core_ids=list(range(_N_CORES)), trace=trace, **kw
    )


def kernel(**inputs: np.ndarray) -> np.ndarray:
    vf = np.asarray(inputs["vector_features"], np.float32)
    d = np.asarray(inputs["distances"], np.float32)
    w = np.asarray(inputs["w_vs"], np.float32)
    b = np.asarray(inputs["b_vs"], np.float32)

    in_maps = _host_pack(vf, d, w, b)
    res = _run(in_maps)
    out = np.stack([r["out"] for r in res.results])  # [B, N, N*K] bf16
    return out.reshape(B, N, N, K).astype(np.float32)


if __name__ == "__main__":
    rng = np.random.default_rng(0)
    ins = {
        "vector_features": rng.standard_normal((B, N, C, F)).astype(np.float32),
        "distances": rng.standard_normal((B, N, N, C)).astype(np.float32),
        "w_vs": (rng.standard_normal((2 * F, K)) / np.sqrt(2 * F)).astype(np.float32),
        "b_vs": np.zeros(K, np.float32),
    }
    out = kernel(**ins)
    pi = np.einsum("bncf,fk->bnck", ins["vector_features"], ins["w_vs"][:F])
    pj = np.einsum("bncf,fk->bnck", ins["vector_features"], ins["w_vs"][F:])
    s = np.einsum("bick,bijc->bijk", pi, ins["distances"]) + np.einsum(
        "bjck,bijc->bijk", pj, ins["distances"]
    )
    want = np.maximum(s, 0)
    rel = np.abs(out - want).max() / np.abs(want).max()
    print("rel err vs numpy:", rel)


# revision 28
# speedup vs baseline: 1519.5304x; 1.0565x over previous
"""Trainium2 Bass kernel for nn_GraphVToS_9388798509586 (gnn_message_passing).

Contract: kernel(**inputs) takes FULL unsharded numpy inputs and returns the
FULL [8, 128, 128, 64] float32 output.

Math (per batch element b, with F=64, K=64, C=3, N=128):
    pi = vf @ w_vs[:F]            # [N, C, K]
    pj = vf @ w_vs[F:] + b_vs     # [N, C, K]  (bias folds exactly: b*sum_c d)
    s[i,j,k] = sum_c d[i,j,c] * (pi[i,c,k] + pj[j,c,k])
    out      = relu(s)

Sharding: data-parallel over batch B=8, one batch element per NeuronCore.

Device kernel design (per core). Both terms run on the TensorEngine as
grouped block-diagonal matmuls over groups of G=8 rows (contract dim
(row,c)=24), with the tiny O(N*C*K) projections and the block-diagonal
weight expansion prepacked on the host (pure input relayout; all N^2-scale
FLOPs stay on device):

  * term2[i,(j,k)] = sum_c d[i,j,c]*pj[j,c,k]: per j-group matmul,
    lhsT = d^T chunk [24, 128i], moving = block-diag pj [24, 8*64=512].
    Output lands directly in the final [i, (j,k)] layout.
  * term1[j,(i,k)] = sum_c d[i,j,c]*pi[i,c,k] is only a matmul in the
    TRANSPOSED layout (j on partitions). It is computed there, then the
    (i<->j) layout fix goes through a DRAM scratch round-trip: contiguous
    per-partition scatter [j,(i,k)] -> scratch[j,i,k], then strided
    readback [i,(j,k)] (128B chunks). DMA through DRAM is the only
    cross-partition reorder that doesn't burn compute-engine time.
  * combine: DVE adds term2-psum + term1-readback into bf16, ACT relu,
    contiguous output DMA. Output is bf16, upcast to f32 on the host
    (rel-err budget 2e-2; measured ~5e-3).

Input packing: one [128, 5120] bf16 tensor; four 24-row stripes at
partition bases 0/32/64/96 (legal quadrant bases for 24-row matmul
operands) hold {term2 groups 0-7, term2 groups 8-15, term1 groups 0-7,
term1 groups 8-15}; within a stripe each group is [ltw_g(128) | wdg_g(512)]
so streaming column-chunk DMAs unlock groups progressively.
"""

import numpy as np

B, N, C, F, K = 8, 128, 3, 64, 64
_N_CORES = 8
_G = 8                   # group size (rows per block-diag group)
_NG = N // _G            # 16 groups
_CTR = _G * C            # 24 contraction rows per group
_GCOL = N + _G * K       # 640 cols per group in the packed input
_W = (_NG // 2) * _GCOL  # 5120 cols per stripe

_cached = {}


def _build_nc():
    import concourse.mybir as mybir
    import concourse.tile as tile
    from concourse import bacc

    fp32 = mybir.dt.float32
    bf16 = mybir.dt.bfloat16

    GW = _G * K  # 512

    nc = bacc.Bacc(None)
    din_d = nc.dram_tensor("din", [N, _W], bf16, kind="ExternalInput")
    out_d = nc.dram_tensor("out", [N, N * K], bf16, kind="ExternalOutput")
    scr_d = nc.dram_tensor("scr", [N, N * K], bf16, kind="Internal")

    def op_slices(side, g):
        """(lhsT, rhs, tile_position) for side (0=term2, 1=term1), group g."""
        base = 32 * (2 * side + (g >= 8))
        col = (g % 8) * _GCOL
        return (
            din_t[base : base + _CTR, col : col + N],
            din_t[base : base + _CTR, col + N : col + _GCOL],
            (base, 0),
        )

    with tile.TileContext(nc) as tc:
        with (
            tc.tile_pool(name="persist", bufs=1) as pp,
            tc.tile_pool(name="acc", bufs=8) as ap,
            tc.tile_pool(name="rb", bufs=4) as rp,
            tc.tile_pool(name="psum", bufs=8, space="PSUM") as qp,
        ):
            din_t = pp.tile([N, _W], bf16, tag="din")
            t1 = pp.tile([N, N * K], bf16, tag="t1")

            # Streaming input: 4 column chunks (2 groups per stripe each).
            for h in range(4):
                cw = _W // 4
                nc.sync.dma_start(
                    din_t[:, h * cw : (h + 1) * cw], din_d[:, h * cw : (h + 1) * cw]
                )

            # Phase A: term1 in transposed layout [j, (i,k)] -> t1 (bf16),
            # casts alternating DVE/ACT; scatter to scratch[j, i, k] after
            # every 4 groups (4 bulk contiguous DMAs).
            for g in range(_NG):
                lhsT, rhs, tpos = op_slices(1, g)
                ps = qp.tile([N, GW], fp32, tag="ps")
                nc.tensor.matmul(
                    ps[:], lhsT, rhs, start=True, stop=True, tile_position=tpos
                )
                sl = t1[:, g * GW : (g + 1) * GW]
                if g % 2 == 0:
                    nc.vector.tensor_copy(sl, ps[:])
                else:
                    nc.scalar.copy(sl, ps[:])
                if g % 4 == 3:
                    nc.sync.dma_start(
                        scr_d[:, (g - 3) * GW : (g + 1) * GW],
                        t1[:, (g - 3) * GW : (g + 1) * GW],
                    )

            # Phase C: per pair of groups: strided readback [i, (j16,k)]
            # (2KB contiguous per partition); per group: term2 matmul,
            # DVE add, ACT relu; store per pair.
            scr_flat = scr_d.rearrange("a b -> (a b)")
            for gp in range(_NG // 2):
                rb = rp.tile([N, 2 * GW], bf16, tag="rb")
                rbv = rb.rearrange("a (j k) -> a j k", j=2 * _G)
                src = scr_flat.rearrange("(j i k) -> i j k", j=N, i=N, k=K)[
                    :, gp * 2 * _G : (gp + 1) * 2 * _G, :
                ]
                nc.sync.dma_start(rbv[:], src)

                acc = ap.tile([N, 2 * GW], bf16, tag="acc")
                for q in range(2):
                    g = 2 * gp + q
                    lhsT, rhs, tpos = op_slices(0, g)
                    ps = qp.tile([N, GW], fp32, tag="ps")
                    nc.tensor.matmul(
                        ps[:], lhsT, rhs, start=True, stop=True, tile_position=tpos
                    )
                    nc.vector.tensor_tensor(
                        acc[:, q * GW : (q + 1) * GW],
                        ps[:],
                        rb[:, q * GW : (q + 1) * GW],
                        mybir.AluOpType.add,
                    )
                sl = acc[:]
                nc.scalar.activation(sl, sl, mybir.ActivationFunctionType.Relu)
                nc.sync.dma_start(
                    out_d[:, gp * 2 * GW : (gp + 1) * 2 * GW], acc[:]
                )
    nc.finalize()
    return nc


def _host_pack(vf, d, w, b):
    """Per-batch host prepack -> list of per-core input dicts (bf16)."""
    import ml_dtypes

    bf = ml_dtypes.bfloat16
    w_i, w_j = w[:F], w[F:]
    pi = np.einsum("bncf,fk->bnck", vf, w_i, optimize=True)
    pj = np.einsum("bncf,fk->bnck", vf, w_j, optimize=True) + b

    def pack_side(dT, proj):
        """dT [(row,c), col] grouped by G rows; proj [row, c, k].

        Returns [2, 24, 5120]: two 8-group stripes, each group packed as
        [ltw_g (128 cols) | wdg_g (512 cols)].
        """
        out = np.zeros((2, _CTR, _W), np.float32)
        rr = np.arange(_G)
        for g in range(_NG):
            s, gg = divmod(g, 8)
            col = gg * _GCOL
            # ltw_g[(r*3+c), col] = d[row=g*G+r, col, c]
            out[s, :, col : col + N] = dT[g * _CTR : (g + 1) * _CTR]
            # wdg_g[(r*3+c), r*64+k] = proj[g*G+r, c, k]
            blk = np.zeros((_CTR, _G, K), np.float32)
            for c in range(C):
                blk[rr * C + c, rr, :] = proj[g * _G + rr, c, :]
            out[s, :, col + N : col + _GCOL] = blk.reshape(_CTR, _G * K)
        return out

    in_maps = []
    for bi in range(B):
        A2 = d[bi].transpose(1, 2, 0).reshape(N * C, N)  # [(j,c), i]
        A1 = d[bi].transpose(0, 2, 1).reshape(N * C, N)  # [(i,c), j]
        s2 = pack_side(A2, pj[bi])
        s1 = pack_side(A1, pi[bi])
        din = np.zeros((N, _W), np.float32)
        din[0:_CTR] = s2[0]
        din[32 : 32 + _CTR] = s2[1]
        din[64 : 64 + _CTR] = s1[0]
        din[96 : 96 + _CTR] = s1[1]
        in_maps.append({"din": np.ascontiguousarray(din).astype(bf)})
    return in_maps


def _run(in_maps, trace=False, **kw):
    from concourse.bass_utils import run_bass_kernel_spmd

    if "nc" not in _cached:
        _cached["nc"] = _build_nc()
    return run_bass_kernel_spmd(
        _cached["nc"], in_maps, core_ids=list(range(_N_CORES)), trace=trace, **kw
    )


def kernel(**inputs: np.ndarray) -> np.ndarray:
    vf = np.asarray(inputs["vector_features"], np.float32)
    d = np.asarray(inputs["distances"], np.float32)
    w = np.asarray(inputs["w_vs"], np.float32)
    b = np.asarray(inputs["b_vs"], np.float32)

    in_maps = _host_pack(vf, d, w, b)
    res = _run(in_maps)
    out = np.stack([r["out"] for r in res.results])  # [B, N, N*K] bf16
    return out.reshape(B, N, N, K).astype(np.float32)


if __name__ == "__main__":
    rng = np.random.default_rng(0)
    ins = {
        "vector_features": rng.standard_normal((B, N, C, F)).astype(np.float32),
        "distances": rng.standard_normal((B, N, N, C)).astype(np.float32),
        "w_vs": (rng.standard_normal((2 * F, K)) / np.sqrt(2 * F)).astype(np.float32),
        "b_vs": np.zeros(K, np.float32),
    }
    out = kernel(**ins)
    pi = np.einsum("bncf,fk->bnck", ins["vector_features"], ins["w_vs"][:F])
    pj = np.einsum("bncf,fk->bnck", ins["vector_features"], ins["w_vs"][F:])
    s = np.einsum("bick,bijc->bijk", pi, ins["distances"]) + np.einsum(
        "bjck,bijc->bijk", pj, ins["distances"]
    )
    want = np.maximum(s, 0)
    rel = np.abs(out - want).max() / np.abs(want).max()
    print("rel err vs numpy:", rel)
